# revision 18
# baseline (speedup 1.0000x reference)
"""Trainium2 Bass kernel for GQA attention (B=2, L=2048, D=4096, H=32, Hk=8,
hd=128) with traditional RoPE and a causal mask, tensor-parallel over 8
NeuronCores.

Sharding: core i owns q-heads 4i..4i+3 (wq cols 512i:512(i+1)) and kv-head i
(wk/wv cols 128i:128(i+1)). Each core projects q/k/v for ALL tokens, applies
RoPE, runs causal attention for its 4 heads, then an AllToAll (split per head
for comm/compute overlap) converts the head-sharded attention output
into token-sharded full-width activations so each core computes
final_out[tokens_i, :] = attn[tokens_i, :] @ wo (full wo). The host
concatenates the 8 token chunks.

Matmul operands are stored in bf16 (f32 PSUM accumulation, ~5e-3 rel err,
fast LDWEIGHTS path, halved DMA/collective traffic); the softmax denominators
ride the AllToAll as an extra row per chunk and normalization happens on the
gathered side, off the attention critical path.
RoPE channel pairs are de-interleaved on the host (even channels -> rows
0..63, odd -> 64..127 of each head tile) so the RoPE "pair swap" becomes a
contiguous 64-partition block swap done with SBUF->SBUF DMA. Softmax
denominators accumulate on the PE (ones-vector matmul into a [1,512] psum
group) so the vector engine stays off the critical path.
"""
import numpy as np
import ml_dtypes

import concourse.bass as bass
import concourse.bacc as bacc
import concourse.tile as tile
import concourse.mybir as mybir
from concourse import bass_utils

F32 = mybir.dt.float32
F32R = mybir.dt.float32r
BF16 = mybir.dt.bfloat16
AF = mybir.ActivationFunctionType

NC_ = 8          # cores
B = 2
L = 2048
D = 4096
H = 32           # q heads
HK = 8           # kv heads
HD = 128         # head dim
HPC = H // NC_   # q heads per core = 4
QC = HPC * HD    # per-core q cols = 512
TOK = B * L      # 4096
TT = 512         # token tile
NTT = TOK // TT  # 8
DT = D // 128    # 32 D tiles
SCALE = float(HD) ** -0.5

_COMPILED = None


def _build():
    nc = bacc.Bacc("TRN2", target_bir_lowering=False, debug=False,
                   num_devices=NC_)

    xT = nc.dram_tensor("xT", [D, TOK], BF16, kind="ExternalInput").ap()
    wq = nc.dram_tensor("wq", [D, QC], BF16, kind="ExternalInput").ap()
    wk = nc.dram_tensor("wk", [D, HD], BF16, kind="ExternalInput").ap()
    wv = nc.dram_tensor("wv", [D, HD], BF16, kind="ExternalInput").ap()
    wo = nc.dram_tensor("wo", [D, D], BF16, kind="ExternalInput").ap()
    ct_d = nc.dram_tensor("ct", [128, L], F32, kind="ExternalInput").ap()
    st_d = nc.dram_tensor("st", [128, L], F32, kind="ExternalInput").ap()
    mb_d = nc.dram_tensor("mb", [128, 4 * TT], BF16, kind="ExternalInput").ap()
    idt_d = nc.dram_tensor("idt", [128, 128], BF16, kind="ExternalInput").ap()
    ones_d = nc.dram_tensor("ones", [128, 1], BF16, kind="ExternalInput").ap()

    out_d = nc.dram_tensor("out", [TT, D], F32, kind="ExternalOutput").ap()
    k_out = nc.dram_tensor("k_out", [128, TOK], BF16, kind="ExternalOutput").ap()
    v_out = nc.dram_tensor("v_out", [128, TOK], BF16, kind="ExternalOutput").ap()

    with tile.TileContext(nc) as tc:
        with (
            tc.tile_pool(name="pers", bufs=1) as pers,
            tc.tile_pool(name="dram", bufs=1, space="DRAM") as dram,
        ):
            # persistent SBUF
            kT_t = pers.tile([128, TOK], BF16, tag="kT")     # roped kT
            v_t = pers.tile([128, TOK], BF16, tag="v")       # v natural, 32 tiles
            ct_t = pers.tile([128, L], F32, tag="ct")
            st_t = pers.tile([128, L], F32, tag="st")
            mb_t = pers.tile([128, 4 * TT], BF16, tag="mb")
            idt_t = pers.tile([128, 128], BF16, tag="idt")
            ones_t = pers.tile([128, 1], BF16, tag="ones")
            nc.scalar.dma_start(ct_t[:], ct_d)
            nc.scalar.dma_start(st_t[:], st_d)
            nc.scalar.dma_start(mb_t[:], mb_d)
            nc.scalar.dma_start(idt_t[:], idt_d)
            nc.scalar.dma_start(ones_t[:], ones_d)

            # DRAM scratch
            qTd = dram.tile([HPC, 128, TOK], BF16, tag="qTd")
            # AllToAll split per head; row 128 of each chunk = denominators
            a2a_in = [dram.tile([NC_, HD + 1, TT], BF16, tag=f"a2ai{p}",
                                name=f"a2ai{p}") for p in range(HPC)]
            a2a_out = [dram.tile([NC_, HD + 1, TT], BF16, tag=f"a2ao{p}",
                                 name=f"a2ao{p}") for p in range(HPC)]
            rdram = [dram.tile([NC_, TT], F32, tag=f"rdram{p}",
                               name=f"rdram{p}") for p in range(HPC)]

            # ---------------- Phase 1: projections + rope ----------------
            with (
                tc.tile_pool(name="wpool", bufs=1) as wpool,
                tc.tile_pool(name="xpool", bufs=10) as xpool,
                tc.tile_pool(name="prj_ps", bufs=1, space="PSUM") as prj_ps,
                tc.tile_pool(name="rpool", bufs=3) as rpool,
            ):
                wq_t = wpool.tile([128, DT, QC], BF16, tag="wq")
                wk_t = wpool.tile([128, DT, HD], BF16, tag="wk")
                wv_t = wpool.tile([128, DT, HD], BF16, tag="wv")
                wq3 = wq.rearrange("(dt p) c -> p dt c", p=128)
                wk3 = wk.rearrange("(dt p) c -> p dt c", p=128)
                wv3 = wv.rearrange("(dt p) c -> p dt c", p=128)

                def rope_from_ps(ps_tile, lsl, dst_f32r_ap):
                    """ps_tile [128, TT] psum F32 (pre-rope). Writes roped F32R
                    to dst_f32r_ap. Uses ST's swap-invariance: swap happens
                    AFTER the sin-multiply, so no psum->sbuf staging copy."""
                    r1 = rpool.tile([128, TT], F32, tag="r1")
                    nc.vector.tensor_mul(r1[:], ps_tile[:], ct_t[:, lsl])
                    r2 = rpool.tile([128, TT], F32, tag="r2")
                    nc.vector.tensor_mul(r2[:], ps_tile[:], st_t[:, lsl])
                    tmp = rpool.tile([128, TT], F32, tag="tmp")
                    nc.sync.dma_start(tmp[0:64, :], r2[64:128, :])
                    nc.sync.dma_start(tmp[64:128, :], r2[0:64, :])
                    nc.vector.tensor_add(dst_f32r_ap, r1[:], tmp[:])

                for tt in range(NTT):
                    ts = slice(tt * TT, (tt + 1) * TT)
                    lsl = slice((tt % (L // TT)) * TT,
                                (tt % (L // TT) + 1) * TT)
                    q_ps = [prj_ps.tile([128, TT], F32, tag=f"q{h}",
                                        name=f"qps{h}",
                                        bufs=2 if h == 0 else 1)
                            for h in range(HPC)]
                    k_ps = prj_ps.tile([128, TT], F32, tag="k")
                    v_ps = prj_ps.tile([128, TT], F32, tag="vv")
                    for dt in range(DT):
                        if tt == 0:
                            nc.sync.dma_start(wq_t[:, dt, :], wq3[:, dt, :])
                            nc.sync.dma_start(wk_t[:, dt, :], wk3[:, dt, :])
                            nc.sync.dma_start(wv_t[:, dt, :], wv3[:, dt, :])
                        xt = xpool.tile([128, TT], BF16, tag="x")
                        nc.sync.dma_start(xt[:], xT[dt * 128:(dt + 1) * 128, ts])
                        st_ = dt == 0
                        sp_ = dt == DT - 1
                        for h in range(HPC):
                            nc.tensor.matmul(
                                q_ps[h][:], wq_t[:, dt, h * HD:(h + 1) * HD],
                                xt[:], start=st_, stop=sp_)
                        nc.tensor.matmul(k_ps[:], wk_t[:, dt, :], xt[:],
                                         start=st_, stop=sp_)
                        nc.tensor.matmul(v_ps[:], wv_t[:, dt, :], xt[:],
                                         start=st_, stop=sp_)
                    vT_sb = rpool.tile([128, TT], BF16, tag="vT")
                    nc.vector.tensor_copy(vT_sb[:], v_ps[:])
                    # rope q -> qTd
                    for h in range(HPC):
                        qro = rpool.tile([128, TT], BF16, tag="qro")
                        rope_from_ps(q_ps[h], lsl, qro[:])
                        nc.sync.dma_start(qTd[h, :, ts], qro[:])
                    # rope k -> kT_t (resident) + k_out
                    rope_from_ps(k_ps, lsl, kT_t[:, ts])
                    nc.sync.dma_start(k_out[:, ts], kT_t[:, ts])
                    # vT -> v_out; transpose -> v_t
                    nc.sync.dma_start(v_out[:, ts], vT_sb[:])
                    for c in range(TT // 128):
                        tp_ps = prj_ps.tile([128, 128], BF16, tag="vv")
                        nc.tensor.transpose(
                            tp_ps[:], vT_sb[:, c * 128:(c + 1) * 128], idt_t[:])
                        j = tt * (TT // 128) + c
                        nc.vector.tensor_copy(
                            v_t[:, j * 128:(j + 1) * 128],
                            tp_ps[:])

            # ---------------- Phase 2: attention (h-outer) ----------------
            with (
                tc.tile_pool(name="qpool", bufs=4) as qpool,
                tc.tile_pool(name="epool", bufs=6) as epool,
                tc.tile_pool(name="npool", bufs=2) as npool,
                tc.tile_pool(name="att_s", bufs=4, space="PSUM") as att_s,
                tc.tile_pool(name="att_av", bufs=2, space="PSUM") as att_av,
                tc.tile_pool(name="att_dn", bufs=2, space="PSUM") as att_dn,
            ):
                for h in range(HPC):
                    for b in range(B):
                        for qj in range(L // TT):
                            tok0 = b * L + qj * TT
                            qt = qpool.tile([128, TT], BF16, tag="qt")
                            nc.sync.dma_start(
                                qt[:], qTd[h, :, tok0:tok0 + TT])
                            av_ps = att_av.tile([128, TT], F32, tag="av")
                            den_ps = att_dn.tile([1, TT], F32, tag="dn")
                            nki = 4 * qj + 4
                            ex_prev = None
                            for ki in range(nki):
                                kg = b * L + ki * 128
                                s_ps = att_s.tile([128, TT], F32, tag="s")
                                nc.tensor.matmul(
                                    s_ps[:], kT_t[:, kg:kg + 128], qt[:],
                                    start=True, stop=True)
                                ex = epool.tile([128, TT], BF16, tag="ex")
                                nc.scalar.activation(ex[:], s_ps[:], AF.Exp,
                                                     scale=SCALE)
                                di = ki - 4 * qj
                                if di >= 0:
                                    nc.vector.tensor_mul(
                                        ex[:], ex[:], mb_t[:, di * TT:(di + 1) * TT])
                                jv = (b * L + ki * 128) // 128
                                nc.tensor.matmul(
                                    av_ps[:], v_t[:, jv * 128:(jv + 1) * 128],
                                    ex[:], start=(ki == 0),
                                    stop=(ki == nki - 1))
                                if ki % 2 == 0:
                                    ex_prev = ex
                                else:
                                    ac2 = epool.tile([128, TT], BF16,
                                                     tag="ac2")
                                    nc.vector.tensor_add(
                                        ac2[:], ex_prev[:], ex[:])
                                    nc.tensor.matmul(
                                        den_ps[:], ones_t[:], ac2[:],
                                        start=(ki == 1),
                                        stop=(ki == nki - 1))
                            ch = b * (L // TT) + qj
                            avu = npool.tile([128, TT], BF16, tag="avu")
                            nc.vector.tensor_copy(avu[:], av_ps[:])
                            nc.sync.dma_start(
                                a2a_in[h][ch, 0:HD, :], avu[:])
                            den_sb = npool.tile([1, TT], BF16, tag="densb")
                            nc.vector.tensor_copy(den_sb[:], den_ps[:])
                            nc.sync.dma_start(
                                a2a_in[h][ch, HD:HD + 1, :], den_sb[:])
                    nc.gpsimd.collective_compute(
                        "AllToAll", mybir.AluOpType.bypass,
                        replica_groups=[list(range(NC_))],
                        ins=[a2a_in[h][:].opt()],
                        outs=[a2a_out[h][:].opt()],
                    )

            # ---------------- Phase 3: output projection ----------------
            # gathered hd row j*128 lives in a2a_out[(j%4)//2]
            #   flat row (j//4)*256 + (j%2)*128
            with (
                tc.tile_pool(name="gpool", bufs=1) as gpool,
                tc.tile_pool(name="wopool", bufs=6) as wopool,
                tc.tile_pool(name="opool", bufs=6) as opool,
                tc.tile_pool(name="wo_ps", bufs=1, space="PSUM") as wo_ps,
            ):
                g_t = gpool.tile([128, DT, TT], BF16, tag="g")
                gflat = [a2a_out[p][:].rearrange("r c t -> (r c) t")
                         for p in range(HPC)]
                RPC = HD + 1  # rows per chunk in a2a buffers
                # j order: head-major waves (j % 4 == p first)
                j_order = [r * 4 + p for p in range(HPC) for r in range(NC_)]
                den8 = [gpool.tile([NC_, TT], BF16, tag=f"den8{p}",
                                   name=f"den8{p}") for p in range(HPC)]
                rec8 = [gpool.tile([NC_, TT], F32, tag=f"rec8{p}",
                                   name=f"rec8{p}") for p in range(HPC)]
                for p in range(HPC):
                    nc.scalar.dma_start(den8[p][:],
                                        a2a_out[p][:, HD:HD + 1, :])
                    nc.vector.reciprocal(rec8[p][:], den8[p][:])
                    nc.scalar.dma_start(rdram[p][:], rec8[p][:])
                    for r in range(NC_):
                        j = r * 4 + p
                        nc.scalar.dma_start(g_t[:, j, :],
                                            gflat[p][r * RPC:r * RPC + 128, :])
                        bcr = wopool.tile([128, TT], F32, tag="bcr")
                        nc.scalar.dma_start(
                            bcr[:], rdram[p][r:r + 1, :]
                            .to_broadcast((128, TT)))
                        nc.vector.tensor_mul(g_t[:, j, :],
                                             g_t[:, j, :], bcr[:])
                for dtp in range(4):
                    dsl = slice(dtp * 1024, (dtp + 1) * 1024)
                    o_ps = [wo_ps.tile([128, TT], F32, tag=f"o{i}",
                                       name=f"ops{i}") for i in range(8)]
                    for jx, j in enumerate(j_order):
                        wo_t = wopool.tile([128, 2 * TT], BF16, tag="wo")
                        nc.sync.dma_start(wo_t[:],
                                          wo[j * 128:(j + 1) * 128, dsl])
                        st_ = jx == 0
                        sp_ = jx == DT - 1
                        for m in range(4):
                            for d2 in range(2):
                                nc.tensor.matmul(
                                    o_ps[m * 2 + d2][:],
                                    g_t[:, j, m * 128:(m + 1) * 128],
                                    wo_t[:, d2 * TT:(d2 + 1) * TT],
                                    start=st_, stop=sp_)
                    for m in range(4):
                        for d2 in range(2):
                            o_sb = opool.tile([128, TT], F32, tag="osb")
                            if d2 == 0:
                                nc.vector.tensor_copy(
                                    o_sb[:], o_ps[m * 2 + d2][:])
                            else:
                                nc.scalar.copy(
                                    o_sb[:], o_ps[m * 2 + d2][:])
                            nc.sync.dma_start(
                                out_d[m * 128:(m + 1) * 128,
                                      dtp * 1024 + d2 * TT:
                                      dtp * 1024 + (d2 + 1) * TT], o_sb[:])

    nc.compile()
    return nc


def _host_prep(x, wq, wk, wv, wo):
    # de-interleave permutation within each 128-wide head block:
    # row r holds channel 2r (r<64) / 2(r-64)+1 (r>=64)
    ch = np.concatenate([np.arange(0, 128, 2), np.arange(1, 128, 2)])

    xT = np.ascontiguousarray(x.reshape(TOK, D).T).astype(ml_dtypes.bfloat16)

    wq_p = wq.reshape(D, H, HD)[:, :, ch].reshape(D, H * HD)
    wk_p = wk.reshape(D, HK, HD)[:, :, ch].reshape(D, HK * HD)

    # rope tables (de-interleaved layout)
    t = np.arange(L, dtype=np.float64)
    freqs = 10000.0 ** (-np.arange(0, HD, 2, dtype=np.float64) / HD)  # [64]
    ang = freqs[:, None] * t[None, :]                                  # [64,L]
    cos = np.cos(ang)
    sin = np.sin(ang)
    ct = np.concatenate([cos, -cos], axis=0).astype(np.float32)  # [128, L]
    st = np.concatenate([sin, sin], axis=0).astype(np.float32)

    # binary mask patterns for the 4 diagonal-crossing k-tiles of a q-tile
    qq = np.arange(TT)
    kk = np.arange(128)
    mb = np.zeros((128, 4 * TT), dtype=np.float32)
    for di in range(4):
        mb[:, di * TT:(di + 1) * TT] = (
            qq[None, :] >= kk[:, None] + 128 * di).astype(np.float32)

    idt = np.eye(128, dtype=np.float32)
    ones = np.ones((128, 1), dtype=np.float32)
    wo_bf = wo.astype(ml_dtypes.bfloat16)

    in_maps = []
    for i in range(NC_):
        in_maps.append({
            "xT": xT,
            "wq": np.ascontiguousarray(
                wq_p[:, i * QC:(i + 1) * QC]).astype(ml_dtypes.bfloat16),
            "wk": np.ascontiguousarray(
                wk_p[:, i * HD:(i + 1) * HD]).astype(ml_dtypes.bfloat16),
            "wv": np.ascontiguousarray(
                wv[:, i * HD:(i + 1) * HD]).astype(ml_dtypes.bfloat16),
            "wo": wo_bf,
            "ct": ct, "st": st, "mb": mb.astype(ml_dtypes.bfloat16),
            "idt": idt.astype(ml_dtypes.bfloat16),
            "ones": ones.astype(ml_dtypes.bfloat16),
        })
    return in_maps, ch


def kernel(x, mask, wq, wk, wv, wo, _trace=False):
    global _COMPILED
    x = np.asarray(x, dtype=np.float32)
    wq = np.asarray(wq, dtype=np.float32)
    wk = np.asarray(wk, dtype=np.float32)
    wv = np.asarray(wv, dtype=np.float32)
    wo = np.asarray(wo, dtype=np.float32)

    in_maps, ch = _host_prep(x, wq, wk, wv, wo)
    if _COMPILED is None:
        _COMPILED = _build()
    nc = _COMPILED

    last_err = None
    for _attempt in range(3):
        try:
            res = bass_utils.run_bass_kernel_spmd(
                nc, in_maps, core_ids=list(range(NC_)), trace=_trace)
            break
        except Exception as e:  # wedged-device retry
            last_err = e
    else:
        raise last_err

    out = np.concatenate([res.results[i]["out"] for i in range(NC_)], axis=0)
    out = out.reshape(B, L, D)

    inv = np.empty(128, dtype=np.int64)
    inv[ch] = np.arange(128)
    k_full = np.empty((B, H, L, HD), dtype=np.float32)
    v_full = np.empty((B, H, L, HD), dtype=np.float32)
    for i in range(NC_):
        kT = np.asarray(res.results[i]["k_out"]).astype(np.float32)[inv]
        vT = np.asarray(res.results[i]["v_out"]).astype(np.float32)
        kb = kT.reshape(HD, B, L).transpose(1, 2, 0)  # [B, L, 128]
        vb = vT.reshape(HD, B, L).transpose(1, 2, 0)
        for j in range(H // HK):
            k_full[:, i * (H // HK) + j] = kb
            v_full[:, i * (H // HK) + j] = vb

    if _trace:
        kernel._last_result = res
    return out, (k_full, v_full)


# revision 19
# speedup vs baseline: 1.0594x; 1.0594x over previous
"""Trainium2 Bass kernel for GQA attention (B=2, L=2048, D=4096, H=32, Hk=8,
hd=128) with traditional RoPE and a causal mask, tensor-parallel over 8
NeuronCores.

Sharding: core i owns q-heads 4i..4i+3 (wq cols 512i:512(i+1)) and kv-head i
(wk/wv cols 128i:128(i+1)). Each core projects q/k/v for ALL tokens, applies
RoPE, runs causal attention for its 4 heads, then an AllToAll (split per head
for comm/compute overlap) converts the head-sharded attention output
into token-sharded full-width activations so each core computes
final_out[tokens_i, :] = attn[tokens_i, :] @ wo (full wo). The host
concatenates the 8 token chunks.

Matmul operands are stored in bf16 (f32 PSUM accumulation, ~5e-3 rel err,
fast LDWEIGHTS path, halved DMA/collective traffic); the softmax denominators
ride the AllToAll as an extra row per chunk and normalization happens on the
gathered side, off the attention critical path.
RoPE channel pairs are de-interleaved on the host (even channels -> rows
0..63, odd -> 64..127 of each head tile) so the RoPE "pair swap" becomes a
contiguous 64-partition block swap done with SBUF->SBUF DMA. Softmax
denominators accumulate on the PE (ones-vector matmul into a [1,512] psum
group) so the vector engine stays off the critical path.
"""
import numpy as np
import ml_dtypes

import concourse.bass as bass
import concourse.bacc as bacc
import concourse.tile as tile
import concourse.mybir as mybir
from concourse import bass_utils

F32 = mybir.dt.float32
F32R = mybir.dt.float32r
BF16 = mybir.dt.bfloat16
AF = mybir.ActivationFunctionType

NC_ = 8          # cores
B = 2
L = 2048
D = 4096
H = 32           # q heads
HK = 8           # kv heads
HD = 128         # head dim
HPC = H // NC_   # q heads per core = 4
QC = HPC * HD    # per-core q cols = 512
TOK = B * L      # 4096
TT = 512         # token tile
NTT = TOK // TT  # 8
DT = D // 128    # 32 D tiles
SCALE = float(HD) ** -0.5

_COMPILED = None


def _build():
    nc = bacc.Bacc("TRN2", target_bir_lowering=False, debug=False,
                   num_devices=NC_)

    xT = nc.dram_tensor("xT", [D, TOK], BF16, kind="ExternalInput").ap()
    wq = nc.dram_tensor("wq", [D, QC], BF16, kind="ExternalInput").ap()
    wk = nc.dram_tensor("wk", [D, HD], BF16, kind="ExternalInput").ap()
    wv = nc.dram_tensor("wv", [D, HD], BF16, kind="ExternalInput").ap()
    wo = nc.dram_tensor("wo", [D, D], BF16, kind="ExternalInput").ap()
    ct_d = nc.dram_tensor("ct", [128, L], F32, kind="ExternalInput").ap()
    st_d = nc.dram_tensor("st", [128, L], F32, kind="ExternalInput").ap()
    mb_d = nc.dram_tensor("mb", [128, 4 * TT], BF16, kind="ExternalInput").ap()
    idt_d = nc.dram_tensor("idt", [128, 128], BF16, kind="ExternalInput").ap()
    ones_d = nc.dram_tensor("ones", [128, 128], BF16, kind="ExternalInput").ap()

    out_d = nc.dram_tensor("out", [TT, D], F32, kind="ExternalOutput").ap()
    k_out = nc.dram_tensor("k_out", [128, TOK], BF16, kind="ExternalOutput").ap()
    v_out = nc.dram_tensor("v_out", [128, TOK], BF16, kind="ExternalOutput").ap()

    with tile.TileContext(nc) as tc:
        with (
            tc.tile_pool(name="pers", bufs=1) as pers,
            tc.tile_pool(name="dram", bufs=1, space="DRAM") as dram,
        ):
            # persistent SBUF
            kT_t = pers.tile([128, TOK], BF16, tag="kT")     # roped kT
            v_t = pers.tile([128, TOK], BF16, tag="v")       # v natural, 32 tiles
            ct_t = pers.tile([128, L], F32, tag="ct")
            st_t = pers.tile([128, L], F32, tag="st")
            mb_t = pers.tile([128, 4 * TT], BF16, tag="mb")
            idt_t = pers.tile([128, 128], BF16, tag="idt")
            ones_t = pers.tile([128, 128], BF16, tag="ones")
            nc.scalar.dma_start(ct_t[:], ct_d)
            nc.scalar.dma_start(st_t[:], st_d)
            nc.scalar.dma_start(mb_t[:], mb_d)
            nc.scalar.dma_start(idt_t[:], idt_d)
            nc.scalar.dma_start(ones_t[:], ones_d)

            # DRAM scratch
            qTd = dram.tile([HPC, 128, TOK], BF16, tag="qTd")
            # AllToAll split per head; row 128 of each chunk = denominators
            a2a_in = [dram.tile([NC_, HD + 1, TT], BF16, tag=f"a2ai{p}",
                                name=f"a2ai{p}") for p in range(HPC)]
            a2a_out = [dram.tile([NC_, HD + 1, TT], BF16, tag=f"a2ao{p}",
                                 name=f"a2ao{p}") for p in range(HPC)]
            rdram = [dram.tile([NC_, TT], F32, tag=f"rdram{p}",
                               name=f"rdram{p}") for p in range(HPC)]

            # ---------------- Phase 1: projections + rope ----------------
            with (
                tc.tile_pool(name="wpool", bufs=1) as wpool,
                tc.tile_pool(name="xpool", bufs=10) as xpool,
                tc.tile_pool(name="prj_ps", bufs=1, space="PSUM") as prj_ps,
                tc.tile_pool(name="rpool", bufs=3) as rpool,
            ):
                wq_t = wpool.tile([128, DT, QC], BF16, tag="wq")
                wk_t = wpool.tile([128, DT, HD], BF16, tag="wk")
                wv_t = wpool.tile([128, DT, HD], BF16, tag="wv")
                wq3 = wq.rearrange("(dt p) c -> p dt c", p=128)
                wk3 = wk.rearrange("(dt p) c -> p dt c", p=128)
                wv3 = wv.rearrange("(dt p) c -> p dt c", p=128)

                def rope_from_ps(ps_tile, lsl, dst_f32r_ap):
                    """ps_tile [128, TT] psum F32 (pre-rope). Writes roped F32R
                    to dst_f32r_ap. Uses ST's swap-invariance: swap happens
                    AFTER the sin-multiply, so no psum->sbuf staging copy."""
                    r1 = rpool.tile([128, TT], F32, tag="r1")
                    nc.vector.tensor_mul(r1[:], ps_tile[:], ct_t[:, lsl])
                    r2 = rpool.tile([128, TT], F32, tag="r2")
                    nc.vector.tensor_mul(r2[:], ps_tile[:], st_t[:, lsl])
                    tmp = rpool.tile([128, TT], F32, tag="tmp")
                    nc.sync.dma_start(tmp[0:64, :], r2[64:128, :])
                    nc.sync.dma_start(tmp[64:128, :], r2[0:64, :])
                    nc.vector.tensor_add(dst_f32r_ap, r1[:], tmp[:])

                for tt in range(NTT):
                    ts = slice(tt * TT, (tt + 1) * TT)
                    lsl = slice((tt % (L // TT)) * TT,
                                (tt % (L // TT) + 1) * TT)
                    q_ps = [prj_ps.tile([128, TT], F32, tag=f"q{h}",
                                        name=f"qps{h}",
                                        bufs=2 if h == 0 else 1)
                            for h in range(HPC)]
                    k_ps = prj_ps.tile([128, TT], F32, tag="k")
                    v_ps = prj_ps.tile([128, TT], F32, tag="vv")
                    for dt in range(DT):
                        if tt == 0:
                            nc.sync.dma_start(wq_t[:, dt, :], wq3[:, dt, :])
                            nc.sync.dma_start(wk_t[:, dt, :], wk3[:, dt, :])
                            nc.sync.dma_start(wv_t[:, dt, :], wv3[:, dt, :])
                        xt = xpool.tile([128, TT], BF16, tag="x")
                        nc.sync.dma_start(xt[:], xT[dt * 128:(dt + 1) * 128, ts])
                        st_ = dt == 0
                        sp_ = dt == DT - 1
                        for h in range(HPC):
                            nc.tensor.matmul(
                                q_ps[h][:], wq_t[:, dt, h * HD:(h + 1) * HD],
                                xt[:], start=st_, stop=sp_)
                        nc.tensor.matmul(k_ps[:], wk_t[:, dt, :], xt[:],
                                         start=st_, stop=sp_)
                        nc.tensor.matmul(v_ps[:], wv_t[:, dt, :], xt[:],
                                         start=st_, stop=sp_)
                    vT_sb = rpool.tile([128, TT], BF16, tag="vT")
                    nc.vector.tensor_copy(vT_sb[:], v_ps[:])
                    # rope q -> qTd
                    for h in range(HPC):
                        qro = rpool.tile([128, TT], BF16, tag="qro")
                        rope_from_ps(q_ps[h], lsl, qro[:])
                        nc.sync.dma_start(qTd[h, :, ts], qro[:])
                    # rope k -> kT_t (resident) + k_out
                    rope_from_ps(k_ps, lsl, kT_t[:, ts])
                    nc.sync.dma_start(k_out[:, ts], kT_t[:, ts])
                    # vT -> v_out; transpose -> v_t
                    nc.sync.dma_start(v_out[:, ts], vT_sb[:])
                    for c in range(TT // 128):
                        tp_ps = prj_ps.tile([128, 128], BF16, tag="vv")
                        nc.tensor.transpose(
                            tp_ps[:], vT_sb[:, c * 128:(c + 1) * 128], idt_t[:])
                        j = tt * (TT // 128) + c
                        nc.vector.tensor_copy(
                            v_t[:, j * 128:(j + 1) * 128],
                            tp_ps[:])

            # ---------------- Phase 2: attention (h-outer) ----------------
            with (
                tc.tile_pool(name="qpool", bufs=4) as qpool,
                tc.tile_pool(name="epool", bufs=6) as epool,
                tc.tile_pool(name="npool", bufs=2) as npool,
                tc.tile_pool(name="att_s", bufs=4, space="PSUM") as att_s,
                tc.tile_pool(name="att_av", bufs=2, space="PSUM") as att_av,
                tc.tile_pool(name="att_dn", bufs=2, space="PSUM") as att_dn,
            ):
                for h in range(HPC):
                    for b in range(B):
                        for qj in range(L // TT):
                            tok0 = b * L + qj * TT
                            qt = qpool.tile([128, TT], BF16, tag="qt")
                            nc.sync.dma_start(
                                qt[:], qTd[h, :, tok0:tok0 + TT])
                            av_ps = att_av.tile([128, TT], F32, tag="av")
                            den_ps = att_dn.tile([128, TT], F32, tag="dn")
                            nki = 4 * qj + 4
                            ex_prev = None
                            for ki in range(nki):
                                kg = b * L + ki * 128
                                s_ps = att_s.tile([128, TT], F32, tag="s")
                                nc.tensor.matmul(
                                    s_ps[:], kT_t[:, kg:kg + 128], qt[:],
                                    start=True, stop=True)
                                ex = epool.tile([128, TT], BF16, tag="ex")
                                nc.scalar.activation(ex[:], s_ps[:], AF.Exp,
                                                     scale=SCALE)
                                di = ki - 4 * qj
                                if di >= 0:
                                    nc.vector.tensor_mul(
                                        ex[:], ex[:], mb_t[:, di * TT:(di + 1) * TT])
                                jv = (b * L + ki * 128) // 128
                                nc.tensor.matmul(
                                    av_ps[:], v_t[:, jv * 128:(jv + 1) * 128],
                                    ex[:], start=(ki == 0),
                                    stop=(ki == nki - 1))
                                if ki % 2 == 0:
                                    ex_prev = ex
                                else:
                                    ac2 = epool.tile([128, TT], BF16,
                                                     tag="ac2")
                                    nc.vector.tensor_add(
                                        ac2[:], ex_prev[:], ex[:])
                                    nc.tensor.matmul(
                                        den_ps[:], ones_t[:], ac2[:],
                                        start=(ki == 1),
                                        stop=(ki == nki - 1))
                            ch = b * (L // TT) + qj
                            avu = npool.tile([128, TT], BF16, tag="avu")
                            nc.vector.tensor_copy(avu[:], av_ps[:])
                            nc.sync.dma_start(
                                a2a_in[h][ch, 0:HD, :], avu[:])
                            den_sb = npool.tile([1, TT], BF16, tag="densb")
                            nc.vector.tensor_copy(den_sb[:], den_ps[0:1, :])
                            nc.sync.dma_start(
                                a2a_in[h][ch, HD:HD + 1, :], den_sb[:])
                    nc.gpsimd.collective_compute(
                        "AllToAll", mybir.AluOpType.bypass,
                        replica_groups=[list(range(NC_))],
                        ins=[a2a_in[h][:].opt()],
                        outs=[a2a_out[h][:].opt()],
                    )

            # ---------------- Phase 3: output projection ----------------
            # gathered hd row j*128 lives in a2a_out[(j%4)//2]
            #   flat row (j//4)*256 + (j%2)*128
            with (
                tc.tile_pool(name="gpool", bufs=1) as gpool,
                tc.tile_pool(name="wopool", bufs=6) as wopool,
                tc.tile_pool(name="opool", bufs=6) as opool,
                tc.tile_pool(name="wo_ps", bufs=1, space="PSUM") as wo_ps,
            ):
                g_t = gpool.tile([128, DT, TT], BF16, tag="g")
                gflat = [a2a_out[p][:].rearrange("r c t -> (r c) t")
                         for p in range(HPC)]
                RPC = HD + 1  # rows per chunk in a2a buffers
                # j order: head-major waves (j % 4 == p first)
                j_order = [r * 4 + p for p in range(HPC) for r in range(NC_)]
                den8 = [gpool.tile([NC_, TT], BF16, tag=f"den8{p}",
                                   name=f"den8{p}") for p in range(HPC)]
                rec8 = [gpool.tile([NC_, TT], F32, tag=f"rec8{p}",
                                   name=f"rec8{p}") for p in range(HPC)]
                for p in range(HPC):
                    nc.scalar.dma_start(den8[p][:],
                                        a2a_out[p][:, HD:HD + 1, :])
                    nc.vector.reciprocal(rec8[p][:], den8[p][:])
                    nc.scalar.dma_start(rdram[p][:], rec8[p][:])
                    for r in range(NC_):
                        j = r * 4 + p
                        nc.scalar.dma_start(g_t[:, j, :],
                                            gflat[p][r * RPC:r * RPC + 128, :])
                        bcr = wopool.tile([128, TT], F32, tag="bcr")
                        nc.scalar.dma_start(
                            bcr[:], rdram[p][r:r + 1, :]
                            .to_broadcast((128, TT)))
                        nc.vector.tensor_mul(g_t[:, j, :],
                                             g_t[:, j, :], bcr[:])
                for dtp in range(4):
                    dsl = slice(dtp * 1024, (dtp + 1) * 1024)
                    o_ps = [wo_ps.tile([128, TT], F32, tag=f"o{i}",
                                       name=f"ops{i}") for i in range(8)]
                    for jx, j in enumerate(j_order):
                        wo_t = wopool.tile([128, 2 * TT], BF16, tag="wo")
                        nc.sync.dma_start(wo_t[:],
                                          wo[j * 128:(j + 1) * 128, dsl])
                        st_ = jx == 0
                        sp_ = jx == DT - 1
                        for m in range(4):
                            for d2 in range(2):
                                nc.tensor.matmul(
                                    o_ps[m * 2 + d2][:],
                                    g_t[:, j, m * 128:(m + 1) * 128],
                                    wo_t[:, d2 * TT:(d2 + 1) * TT],
                                    start=st_, stop=sp_)
                    for m in range(4):
                        for d2 in range(2):
                            o_sb = opool.tile([128, TT], F32, tag="osb")
                            if d2 == 0:
                                nc.vector.tensor_copy(
                                    o_sb[:], o_ps[m * 2 + d2][:])
                            else:
                                nc.scalar.copy(
                                    o_sb[:], o_ps[m * 2 + d2][:])
                            nc.sync.dma_start(
                                out_d[m * 128:(m + 1) * 128,
                                      dtp * 1024 + d2 * TT:
                                      dtp * 1024 + (d2 + 1) * TT], o_sb[:])

    nc.compile()
    return nc


def _host_prep(x, wq, wk, wv, wo):
    # de-interleave permutation within each 128-wide head block:
    # row r holds channel 2r (r<64) / 2(r-64)+1 (r>=64)
    ch = np.concatenate([np.arange(0, 128, 2), np.arange(1, 128, 2)])

    xT = np.ascontiguousarray(x.reshape(TOK, D).T).astype(ml_dtypes.bfloat16)

    wq_p = wq.reshape(D, H, HD)[:, :, ch].reshape(D, H * HD)
    wk_p = wk.reshape(D, HK, HD)[:, :, ch].reshape(D, HK * HD)

    # rope tables (de-interleaved layout)
    t = np.arange(L, dtype=np.float64)
    freqs = 10000.0 ** (-np.arange(0, HD, 2, dtype=np.float64) / HD)  # [64]
    ang = freqs[:, None] * t[None, :]                                  # [64,L]
    cos = np.cos(ang)
    sin = np.sin(ang)
    ct = np.concatenate([cos, -cos], axis=0).astype(np.float32)  # [128, L]
    st = np.concatenate([sin, sin], axis=0).astype(np.float32)

    # binary mask patterns for the 4 diagonal-crossing k-tiles of a q-tile
    qq = np.arange(TT)
    kk = np.arange(128)
    mb = np.zeros((128, 4 * TT), dtype=np.float32)
    for di in range(4):
        mb[:, di * TT:(di + 1) * TT] = (
            qq[None, :] >= kk[:, None] + 128 * di).astype(np.float32)

    idt = np.eye(128, dtype=np.float32)
    ones = np.ones((128, 128), dtype=np.float32)
    wo_bf = wo.astype(ml_dtypes.bfloat16)

    in_maps = []
    for i in range(NC_):
        in_maps.append({
            "xT": xT,
            "wq": np.ascontiguousarray(
                wq_p[:, i * QC:(i + 1) * QC]).astype(ml_dtypes.bfloat16),
            "wk": np.ascontiguousarray(
                wk_p[:, i * HD:(i + 1) * HD]).astype(ml_dtypes.bfloat16),
            "wv": np.ascontiguousarray(
                wv[:, i * HD:(i + 1) * HD]).astype(ml_dtypes.bfloat16),
            "wo": wo_bf,
            "ct": ct, "st": st, "mb": mb.astype(ml_dtypes.bfloat16),
            "idt": idt.astype(ml_dtypes.bfloat16),
            "ones": ones.astype(ml_dtypes.bfloat16),
        })
    return in_maps, ch


def kernel(x, mask, wq, wk, wv, wo, _trace=False):
    global _COMPILED
    x = np.asarray(x, dtype=np.float32)
    wq = np.asarray(wq, dtype=np.float32)
    wk = np.asarray(wk, dtype=np.float32)
    wv = np.asarray(wv, dtype=np.float32)
    wo = np.asarray(wo, dtype=np.float32)

    in_maps, ch = _host_prep(x, wq, wk, wv, wo)
    if _COMPILED is None:
        _COMPILED = _build()
    nc = _COMPILED

    last_err = None
    for _attempt in range(3):
        try:
            res = bass_utils.run_bass_kernel_spmd(
                nc, in_maps, core_ids=list(range(NC_)), trace=_trace)
            break
        except Exception as e:  # wedged-device retry
            last_err = e
    else:
        raise last_err

    out = np.concatenate([res.results[i]["out"] for i in range(NC_)], axis=0)
    out = out.reshape(B, L, D)

    inv = np.empty(128, dtype=np.int64)
    inv[ch] = np.arange(128)
    k_full = np.empty((B, H, L, HD), dtype=np.float32)
    v_full = np.empty((B, H, L, HD), dtype=np.float32)
    for i in range(NC_):
        kT = np.asarray(res.results[i]["k_out"]).astype(np.float32)[inv]
        vT = np.asarray(res.results[i]["v_out"]).astype(np.float32)
        kb = kT.reshape(HD, B, L).transpose(1, 2, 0)  # [B, L, 128]
        vb = vT.reshape(HD, B, L).transpose(1, 2, 0)
        for j in range(H // HK):
            k_full[:, i * (H // HK) + j] = kb
            v_full[:, i * (H // HK) + j] = vb

    if _trace:
        kernel._last_result = res
    return out, (k_full, v_full)
